# revision 66
# baseline (speedup 1.0000x reference)
"""Trainium2 Bass kernel for nn_Ani_layer (dense_cnn).

A 64->64ch 3x3 conv whose weight is built from params x basis, with
per-window mean subtraction folded into the conv weights, a vector-norm
"relu" epilogue (out/norm masked where norm<=b) and mean re-add.

Distribution: 8 shards = (batch b in 0..3) x (H half in 0..1); each core
gets a pre-padded bf16 (64ch, 66, 130) input slab and produces
(64ch, 64, 128) bf16 (host converts to fp32). No collectives.

v3 structure (vs the earlier 5-matmul/xb baseline):
  - Single SBUF x buffer xt = [x ; x shifted down one row] (128 part,
    67, 130; 2.2MB HBM instead of 4.3MB). Taps (0,j)/(1,j) pair into 3
    contract-128 matmuls; taps (2,j) are 3 contract-64 matmuls from the
    lower half. Mean rows (psum partitions 64-127) fold into the same
    stationary weights.
  - Super-batches: 2 singles, 5x 3-row-group triples, then a tapering
    tail (pair + 3 singles) keeping post-conv drain chains short. Two
    psum pools (2x3-bank + 2x1-bank tiles). The psum tile's ONLY
    readers are two ACT ops right after conv, so the PE stream is
    fully decoupled from the epilogue: it runs HAM-warm at 2.4GHz
    (~166ns per N=390 matmul) and finishes all 132 matmuls early.
  - Epilogue per SB: ACT cb=Identity(psum+bias) evacuates conv AND avg
    rows in one 128-partition op (bf16); ACT sq1=Square(psum comp1);
    DVE custom op n2m=select(sq(cb0)+sq1>b^2, ., BIG) (all-SBUF bf16,
    base-partition-0 operands -- the DVE two-SBUF-input base-equality
    rule is load-bearing); ACT Rsqrt LUT (raw emission); DVE dup; DVE
    m=cb*r written at partitions 64-127 so the Pool-engine merge
    (avg+m, tensor_add) sees both inputs at base 64 (DVE handles the
    tail merges); bf16 output DMA per SB (fp32 conversion on host).
  - Stage split: stage_b(k-1) emitted AFTER stage_a(k) so rsqrt never
    head-blocks the strict-FIFO ACT queue.
  - DMAs: weight blob as two parallel partition-halves (SP+ACT
    queues), x chunks interleaved lo->SP / hi->ACT, outputs on SP.
"""

import os
import sys
from contextlib import ExitStack

for _p in ("/opt/trn_rl_repo", os.path.expanduser("~/.axon_site/_ro/trn_rl_repo")):
    if os.path.isdir(_p) and _p not in sys.path:
        sys.path.insert(0, _p)

import numpy as np
import ml_dtypes

import concourse.bass as bass
import concourse.bacc as bacc
import concourse.tile as tile
import concourse.dve_ops as dve_ops_mod
from concourse import mybir
from concourse.bass_utils import run_bass_kernel_spmd
from concourse.dve_spec import C0, C1, C2, Spec, Src0, Src1, lower, select, sq
from concourse.dve_spec import _has_src1
from concourse.dve_uop import DveOpSpec

F32 = mybir.dt.float32
BF16 = mybir.dt.bfloat16
ALU = mybir.AluOpType
ACTF = mybir.ActivationFunctionType

B, O, I, KS, H, W = 4, 32, 32, 3, 128, 128
NCH = 2 * I          # 64 input channels
HS = H // 2          # 64 output rows per shard
PH, PW = HS + 2, W + 2   # padded shard: 66 x 130
N_CORES = 8
BIG = 1.0e12         # masked pixels: n2 -> BIG so Rsqrt(BIG) ~ 1e-6 ~ 0
NBLOB = 3 * 128 + 3 * 128    # wp(3) + wv(3)


def _register_dve_op(name, spec):
    for op in dve_ops_mod.OPS:
        if op.name == name:
            return op
    row = dve_ops_mod._CUSTOM_DVE_ROW_BASE + len(dve_ops_mod.OPS)
    assert row < 0x20
    dve_ops_mod._SUB_OPCODE_FOR_NAME[name] = row
    uops = lower(spec, ver="v3")
    sha = DveOpSpec(name=name, opcode=row, uops=uops,
                    rd1_en=_has_src1(spec)).sha("v3")
    op = dve_ops_mod.DveOp(name, spec, subdim=False, uops_sha={"v3": sha})
    dve_ops_mod.OPS.append(op)
    dve_ops_mod.CUSTOM_DVE_SPECS[name] = spec
    return op


def _sqsum_sel_op():
    # x = (conv0 + bias0)^2 + (pre-squared t1); sq() on BOTH inputs hangs
    # the DVE, so in1 arrives already squared. C0 = per-partition bias,
    # C1 = b^2, C2 (imm2) = BIG for masked pixels.
    x = sq(Src0 + C0) + Src1
    body = select(x > C1, x, C2)

    def ref(in0, in1, c0, c1, c2):
        xx = (in0.astype(np.float32) + c0) ** 2 + in1.astype(np.float32)
        return np.where(xx > c1, xx, c2)

    return _register_dve_op("SQB_SEL_ANT", Spec(body=body, reference=ref))


def _act_raw(nc, out, in_, func, bias_ap, scale):
    """Emit InstActivation directly (bass bans Rsqrt; the reciprocal_sqrt
    LUT measures ~5e-5 max rel err over [1e-4, 1e2])."""
    eng = nc.scalar
    inputs = [eng.lower_ap(in_), eng.lower_ap(bias_ap),
              mybir.ImmediateValue(dtype=mybir.dt.float32, value=scale),
              mybir.ImmediateValue(dtype=mybir.dt.float32, value=0.0)]
    return eng.add_instruction(mybir.InstActivation(
        name=nc.get_next_instruction_name(), func=func,
        ins=inputs, outs=[eng.lower_ap(out)]))


_NC = {}

# super-batches: two singles first (early epilogue start), five 3-group
# batches (low per-op overhead), then a tapering tail (pair + three
# singles) so the drain chains after the last conv stay short
GROUPS = [(3 * i, 3) for i in range(21)] + [(63, 1)]
SBS = [[GROUPS[0]], [GROUPS[1]]] \
    + [GROUPS[2 + 3 * i:5 + 3 * i] for i in range(5)] \
    + [[GROUPS[17], GROUPS[18]], [GROUPS[19], GROUPS[20]], [GROUPS[21]]]


def _build_nc(b2):
    op_sqsum = _sqsum_sel_op()

    nc = bacc.Bacc("TRN2")
    x_d = nc.declare_dram_parameter("x", [NCH, PH, PW], BF16, isOutput=False)
    wb_d = nc.declare_dram_parameter("wb", [128, NBLOB], BF16, isOutput=False)
    cst_d = nc.declare_dram_parameter("cst", [128, 2], F32, isOutput=False)
    out_d = nc.declare_dram_parameter("out", [NCH, HS * W], BF16, isOutput=True)

    with tile.TileContext(nc) as tc, ExitStack() as ctx:
        singles = ctx.enter_context(tc.tile_pool(name="singles", bufs=1))
        psum = ctx.enter_context(tc.tile_pool(name="psum", bufs=2, space="PSUM"))
        psum1 = ctx.enter_context(tc.tile_pool(name="psum1", bufs=2,
                                               space="PSUM"))
        ep = ctx.enter_context(tc.tile_pool(name="ep", bufs=8))
        mpool = ctx.enter_context(tc.tile_pool(name="mpool", bufs=8))
        outp = ctx.enter_context(tc.tile_pool(name="outp", bufs=8))

        xt = singles.tile([128, PH + 1, PW], BF16, tag="xt")
        wb = singles.tile([128, NBLOB], BF16, tag="wb")
        cst = singles.tile([128, 2], F32, tag="cst")
        zb = singles.tile([O, 1], F32, tag="zb")
        nc.vector.memset(zb, 0.0)

        # weights first (both queues in parallel, half each); the first
        # 128 columns (weight w0) go as a separate tiny DMA so conv(0)'s
        # first LDWEIGHTS can start ~1us earlier
        nc.sync.dma_start(out=wb[0:NCH, 0:128], in_=wb_d[0:NCH, 0:128])
        nc.scalar.dma_start(out=wb[NCH:128, 0:128], in_=wb_d[NCH:128, 0:128])
        nc.sync.dma_start(out=wb[0:NCH, 128:NBLOB], in_=wb_d[0:NCH, 128:NBLOB])
        nc.scalar.dma_start(out=wb[NCH:128, 128:NBLOB],
                            in_=wb_d[NCH:128, 128:NBLOB])

        # x load: lower = x rows 0..65, upper = x shifted down one row
        # (rows 0..64 <- x rows 1..65). Chunked so conv(0) starts early;
        # issue alternates SP / Scalar to halve serial issue latency.
        CHUNKS = [(0, 6), (6, 16), (16, 28), (28, 46), (46, 66)]
        for ci, (r0, r1) in enumerate(CHUNKS):
            nc.sync.dma_start(out=xt[0:NCH, r0:r1, :], in_=x_d[:, r0:r1, :])
            r1b = min(PH - 1, r1)
            if r1b > r0:
                nc.scalar.dma_start(out=xt[NCH:128, r0:r1b, :],
                                    in_=x_d[:, r0 + 1:r1b + 1, :])
            if ci == 0:
                nc.scalar.dma_start(out=cst, in_=cst_d[:, :])

        xtf = xt.rearrange("p a b -> p (a b)")

        def conv(k):
            sb = SBS[k]
            if len(sb) == 1:
                pt = psum1.tile([128, 1, 512], F32, tag="pt1")
            else:
                pt = psum.tile([128, 3, 512], F32, tag="pt")
            # weight-major: taps (0,j)+(1,j) contract-128, then (2,j)
            # contract-64; one stationary load per weight.
            for j in range(3):
                for t, (h0, nr) in enumerate(sb):
                    nc.tensor.matmul(pt[:, t, 0:nr * PW],
                                     wb[:, 128 * j:128 * (j + 1)],
                                     xtf[:, h0 * PW + j:h0 * PW + j + nr * PW],
                                     start=(j == 0), stop=False)
            for j in range(3):
                for t, (h0, nr) in enumerate(sb):
                    nc.tensor.matmul(pt[:, t, 0:nr * PW],
                                     wb[0:NCH, 128 * (3 + j):128 * (4 + j)],
                                     xtf[0:NCH,
                                         (h0 + 2) * PW + j:
                                         (h0 + 2) * PW + j + nr * PW],
                                     start=False, stop=(j == 2))
            return pt

        held = {}

        def stage_a(k, pt):
            sb = SBS[k]
            tc_n = len(sb)
            NW = sb[0][1] * PW       # 390 or 130 (singles)
            # cb = (conv + bias ; avg) -> bf16, one 128-partition ACT op
            cb = ep.tile([128, tc_n, NW], BF16, tag="cb")
            nc.scalar.activation(cb, pt[:, 0:tc_n, 0:NW], ACTF.Identity,
                                 bias=cst[:, 0:1], scale=1.0)
            # sq1 = (conv1 + bias1)^2 straight from PSUM
            sq_t = ep.tile([O, tc_n, NW], BF16, tag="sq")
            nc.scalar.activation(sq_t, pt[O:NCH, 0:tc_n, 0:NW], ACTF.Square,
                                 bias=cst[O:NCH, 0:1], scale=1.0)
            # n2m = select((conv0+bias0)^2 + sq1 > b^2, ., BIG)
            n2_t = ep.tile([O, tc_n, NW], BF16, tag="n2")
            nc.vector._custom_dve(
                op_sqsum,
                out=n2_t.rearrange("p a b -> p (a b)"),
                in0=cb[0:O].rearrange("p a b -> p (a b)"),
                in1=sq_t.rearrange("p a b -> p (a b)"),
                s0=0.0, s1=b2, imm2=BIG)
            held[k] = (cb, n2_t)

        def stage_b(k, pt):
            sb = SBS[k]
            tc_n = len(sb)
            NW = sb[0][1] * PW
            cb, n2_t = held.pop(k)
            # r = 1/sqrt(n2m), duplicated to partitions 32-63
            r_t = ep.tile([NCH, tc_n, NW], BF16, tag="r")
            _act_raw(nc, r_t[0:O], n2_t, ACTF.Rsqrt, zb, 1.0)
            nc.vector.tensor_copy(r_t[O:NCH], r_t[0:O])
            # m = cb * r, written to partitions 64-127 so the merge sees
            # both inputs at base partition 64
            m_t = mpool.tile([128, tc_n, NW], BF16, tag="m")
            nc.vector.tensor_mul(m_t[NCH:128], cb[0:NCH], r_t)
            # out = avg + m (Pool mid-kernel; DVE for the last tiny singles)
            ot = outp.tile([NCH, tc_n, NW], BF16, tag="ot")
            eng = nc.gpsimd if k < NSB - 2 else nc.vector
            eng.tensor_add(ot, cb[NCH:128], m_t[NCH:128])
            h0 = sb[0][0]
            nr = sb[0][1]
            tot = tc_n * nr * W
            nc.sync.dma_start(
                out=out_d[:, h0 * W:h0 * W + tot].rearrange(
                    "p (a r c) -> p a r c", a=tc_n, r=nr),
                in_=ot.rearrange("p a (r c) -> p a r c", c=PW)[:, :, :, 0:W])

        NSB = len(SBS)
        pts = [conv(0)]
        for bi in range(NSB):
            if bi + 1 < NSB:
                pts.append(conv(bi + 1))
            stage_a(bi, pts[bi])
            # stage_b of the PREVIOUS batch after this batch's stage_a: its
            # rsqrt input (n2m) is then long ready, so ACT never head-blocks
            if bi >= 1:
                stage_b(bi - 1, pts[bi - 1])
        stage_b(NSB - 1, pts[NSB - 1])

    nc.compile()
    return nc


def _get_nc(b2):
    key = float(b2)
    if key not in _NC:
        _NC[key] = _build_nc(key)
    return _NC[key]


def _prep(params, basis, bias_term, b):
    params = np.asarray(params, np.float32)
    basis = np.asarray(basis, np.float32)
    Kr = np.einsum("abcd,cdefgh->abefgh", params, basis)  # (O,I,K,K,2,2)
    kern = Kr.transpose(0, 4, 1, 5, 2, 3).reshape(2 * O, 2 * I, KS, KS)
    # reference pairs patch (kh=q, kw=p) with kern[o2, c, p, q]:
    Wtap = kern.transpose(0, 1, 3, 2)  # [o2, c, dh, dw]
    # fold per-window mean subtraction into the weights
    Ksum = np.stack([Wtap[:, 0::2].sum(axis=(1, 2, 3)),
                     Wtap[:, 1::2].sum(axis=(1, 2, 3))], axis=1)  # [o2, 2]
    cpar = np.arange(NCH) % 2
    Wp = Wtap - (Ksum[:, cpar] / float(I * KS * KS))[:, :, None, None]
    # device output order: dev channel = 32*v + o  <->  torch channel 2*o + v
    perm = np.array([2 * (i % O) + i // O for i in range(NCH)])
    # avg weights: dev avg row 32*v + o' sums input channels of parity v
    avgw = np.zeros((NCH, NCH), np.float32)   # [avg out row, in ch]
    for v in (0, 1):
        avgw[O * v:O * v + O, v::2] = 1.0 / float(I * KS * KS)
    # blob: [contract row, 6*128 weight cols + 64 id cols]
    blob = np.zeros((128, NBLOB), np.float32)
    for j in range(3):
        # contract-128 weights: row c + 64*s -> tap (s, j)
        for s in (0, 1):
            blob[NCH * s:NCH * s + NCH, 128 * j:128 * j + NCH] = \
                Wp[perm, :, s, j].T
            blob[NCH * s:NCH * s + NCH, 128 * j + NCH:128 * j + 128] = avgw.T
        # contract-64 weights: row c -> tap (2, j)
        blob[0:NCH, 128 * (3 + j):128 * (3 + j) + NCH] = Wp[perm, :, 2, j].T
        blob[0:NCH, 128 * (3 + j) + NCH:128 * (3 + j) + 128] = avgw.T
    bt = np.asarray(bias_term, np.float32).reshape(O, 2)
    cst = np.zeros((128, 2), np.float32)
    for v in (0, 1):
        cst[O * v:O * v + O, 0] = bt[:, v]
    cst[0:O, 1] = bt[:, 1]
    b2 = float(np.asarray(b).reshape(-1)[0]) ** 2
    return blob.astype(ml_dtypes.bfloat16), cst, b2, perm


def _run(inputs, trace=False):
    xx = np.asarray(inputs["xx"], np.float32)
    blob, cst, b2, perm = _prep(inputs["params"], inputs["basis"],
                                inputs["bias_term"], inputs["b"])
    xp = np.pad(xx, ((0, 0), (0, 0), (1, 1), (1, 1)), mode="edge")
    xpb = xp.astype(ml_dtypes.bfloat16)
    in_maps = []
    for core in range(N_CORES):
        bb, half = core // 2, core % 2
        shard = np.ascontiguousarray(xpb[bb, :, half * HS:half * HS + PH, :])
        in_maps.append({"x": shard, "wb": blob, "cst": cst})
    nc = _get_nc(b2)
    res = run_bass_kernel_spmd(nc, in_maps, list(range(N_CORES)), trace=trace)
    out = np.zeros((B, NCH, H, W), np.float32)
    for core in range(N_CORES):
        bb, half = core // 2, core % 2
        dev = np.asarray(res.results[core]["out"]).astype(np.float32)
        out[bb, perm, half * HS:(half + 1) * HS, :] = dev.reshape(NCH, HS, W)
    return out, res.exec_time_ns


def kernel(**inputs):
    out, _ = _run(inputs, trace=False)
    return out


# revision 67
# speedup vs baseline: 1.0320x; 1.0320x over previous
"""Trainium2 Bass kernel for nn_Ani_layer (dense_cnn).

A 64->64ch 3x3 conv whose weight is built from params x basis, with
per-window mean subtraction folded into the conv weights, a vector-norm
"relu" epilogue (out/norm masked where norm<=b) and mean re-add.

Distribution: 8 shards = (batch b in 0..3) x (H half in 0..1); each core
gets a pre-padded bf16 (64ch, 66, 130) input slab and produces
(64ch, 64, 128) bf16 (host converts to fp32). No collectives.

v3 structure (vs the earlier 5-matmul/xb baseline):
  - Single SBUF x buffer xt = [x ; x shifted down one row] (128 part,
    67, 130; 2.2MB HBM instead of 4.3MB). Taps (0,j)/(1,j) pair into 3
    contract-128 matmuls; taps (2,j) are 3 contract-64 matmuls from the
    lower half. Mean rows (psum partitions 64-127) fold into the same
    stationary weights.
  - Super-batches: 2 singles, 5x 3-row-group triples, then a tapering
    tail (pair + 3 singles) keeping post-conv drain chains short. Two
    psum pools (2x3-bank + 2x1-bank tiles). The psum tile's ONLY
    readers are two ACT ops right after conv, so the PE stream is
    fully decoupled from the epilogue: it runs HAM-warm at 2.4GHz
    (~166ns per N=390 matmul) and finishes all 132 matmuls early.
  - Epilogue per SB: ACT cb=Identity(psum+bias) evacuates conv AND avg
    rows in one 128-partition op (bf16); ACT sq1=Square(psum comp1);
    DVE custom op n2m=select(sq(cb0)+sq1>b^2, ., BIG) (all-SBUF bf16,
    base-partition-0 operands -- the DVE two-SBUF-input base-equality
    rule is load-bearing); ACT Rsqrt LUT (raw emission); DVE dup; DVE
    m=cb*r written at partitions 64-127 so the Pool-engine merge
    (avg+m, tensor_add) sees both inputs at base 64 (DVE handles the
    tail merges); bf16 output DMA per SB (fp32 conversion on host).
  - Stage split: stage_b(k-1) emitted AFTER stage_a(k) so rsqrt never
    head-blocks the strict-FIFO ACT queue.
  - DMAs: weight blob as two parallel partition-halves (SP+ACT
    queues), x chunks interleaved lo->SP / hi->ACT, outputs on SP.
"""

import os
import sys
from contextlib import ExitStack

for _p in ("/opt/trn_rl_repo", os.path.expanduser("~/.axon_site/_ro/trn_rl_repo")):
    if os.path.isdir(_p) and _p not in sys.path:
        sys.path.insert(0, _p)

import numpy as np
import ml_dtypes

import concourse.bass as bass
import concourse.bacc as bacc
import concourse.tile as tile
import concourse.dve_ops as dve_ops_mod
from concourse import mybir
from concourse.bass_utils import run_bass_kernel_spmd
from concourse.dve_spec import C0, C1, C2, Spec, Src0, Src1, lower, select, sq
from concourse.dve_spec import _has_src1
from concourse.dve_uop import DveOpSpec

F32 = mybir.dt.float32
BF16 = mybir.dt.bfloat16
ALU = mybir.AluOpType
ACTF = mybir.ActivationFunctionType

B, O, I, KS, H, W = 4, 32, 32, 3, 128, 128
NCH = 2 * I          # 64 input channels
HS = H // 2          # 64 output rows per shard
PH, PW = HS + 2, W + 2   # padded shard: 66 x 130
N_CORES = 8
BIG = 1.0e12         # masked pixels: n2 -> BIG so Rsqrt(BIG) ~ 1e-6 ~ 0
NBLOB = 3 * 128 + 3 * 128    # wp(3) + wv(3)


def _register_dve_op(name, spec):
    for op in dve_ops_mod.OPS:
        if op.name == name:
            return op
    row = dve_ops_mod._CUSTOM_DVE_ROW_BASE + len(dve_ops_mod.OPS)
    assert row < 0x20
    dve_ops_mod._SUB_OPCODE_FOR_NAME[name] = row
    uops = lower(spec, ver="v3")
    sha = DveOpSpec(name=name, opcode=row, uops=uops,
                    rd1_en=_has_src1(spec)).sha("v3")
    op = dve_ops_mod.DveOp(name, spec, subdim=False, uops_sha={"v3": sha})
    dve_ops_mod.OPS.append(op)
    dve_ops_mod.CUSTOM_DVE_SPECS[name] = spec
    return op


def _sqsum_sel_op():
    # x = (conv0 + bias0)^2 + (pre-squared t1); sq() on BOTH inputs hangs
    # the DVE, so in1 arrives already squared. C0 = per-partition bias,
    # C1 = b^2, C2 (imm2) = BIG for masked pixels.
    x = sq(Src0 + C0) + Src1
    body = select(x > C1, x, C2)

    def ref(in0, in1, c0, c1, c2):
        xx = (in0.astype(np.float32) + c0) ** 2 + in1.astype(np.float32)
        return np.where(xx > c1, xx, c2)

    return _register_dve_op("SQB_SEL_ANT", Spec(body=body, reference=ref))


def _act_raw(nc, out, in_, func, bias_ap, scale):
    """Emit InstActivation directly (bass bans Rsqrt; the reciprocal_sqrt
    LUT measures ~5e-5 max rel err over [1e-4, 1e2])."""
    eng = nc.scalar
    inputs = [eng.lower_ap(in_), eng.lower_ap(bias_ap),
              mybir.ImmediateValue(dtype=mybir.dt.float32, value=scale),
              mybir.ImmediateValue(dtype=mybir.dt.float32, value=0.0)]
    return eng.add_instruction(mybir.InstActivation(
        name=nc.get_next_instruction_name(), func=func,
        ins=inputs, outs=[eng.lower_ap(out)]))


_NC = {}

# super-batches: two singles first (early epilogue start), five 3-group
# batches (low per-op overhead), then a tapering tail (pair + three
# singles) so the drain chains after the last conv stay short
GROUPS = [(3 * i, 3) for i in range(21)] + [(63, 1)]
SBS = [[GROUPS[0]], [GROUPS[1]]] \
    + [GROUPS[2 + 3 * i:5 + 3 * i] for i in range(5)] \
    + [[GROUPS[17], GROUPS[18]], [GROUPS[19], GROUPS[20]], [GROUPS[21]]]


def _build_nc(b2):
    op_sqsum = _sqsum_sel_op()

    nc = bacc.Bacc("TRN2")
    x_d = nc.declare_dram_parameter("x", [NCH, PH, PW], BF16, isOutput=False)
    wb_d = nc.declare_dram_parameter("wb", [128, NBLOB], BF16, isOutput=False)
    cst_d = nc.declare_dram_parameter("cst", [128, 2], F32, isOutput=False)
    out_d = nc.declare_dram_parameter("out", [NCH, HS * W], BF16, isOutput=True)

    with tile.TileContext(nc) as tc, ExitStack() as ctx:
        singles = ctx.enter_context(tc.tile_pool(name="singles", bufs=1))
        psum = ctx.enter_context(tc.tile_pool(name="psum", bufs=2, space="PSUM"))
        psum1 = ctx.enter_context(tc.tile_pool(name="psum1", bufs=2,
                                               space="PSUM"))
        ep = ctx.enter_context(tc.tile_pool(name="ep", bufs=8))
        mpool = ctx.enter_context(tc.tile_pool(name="mpool", bufs=8))
        outp = ctx.enter_context(tc.tile_pool(name="outp", bufs=8))

        xt = singles.tile([128, PH + 1, PW], BF16, tag="xt")
        wb = singles.tile([128, NBLOB], BF16, tag="wb")
        cst = singles.tile([128, 2], F32, tag="cst")
        zb = singles.tile([O, 1], F32, tag="zb")
        nc.vector.memset(zb, 0.0)

        # weights first (both queues in parallel, half each), then x chunks
        nc.sync.dma_start(out=wb[0:NCH], in_=wb_d[0:NCH, :])
        nc.scalar.dma_start(out=wb[NCH:128], in_=wb_d[NCH:128, :])

        # x load: lower = x rows 0..65, upper = x shifted down one row
        # (rows 0..64 <- x rows 1..65). Chunked so conv(0) starts early;
        # issue alternates SP / Scalar to halve serial issue latency.
        CHUNKS = [(0, 6), (6, 16), (16, 28), (28, 46), (46, 66)]
        for ci, (r0, r1) in enumerate(CHUNKS):
            nc.sync.dma_start(out=xt[0:NCH, r0:r1, :], in_=x_d[:, r0:r1, :])
            r1b = min(PH - 1, r1)
            if r1b > r0:
                nc.scalar.dma_start(out=xt[NCH:128, r0:r1b, :],
                                    in_=x_d[:, r0 + 1:r1b + 1, :])
            if ci == 0:
                nc.scalar.dma_start(out=cst, in_=cst_d[:, :])

        xtf = xt.rearrange("p a b -> p (a b)")

        def conv(k):
            sb = SBS[k]
            if len(sb) == 1:
                pt = psum1.tile([128, 1, 512], F32, tag="pt1")
            else:
                pt = psum.tile([128, 3, 512], F32, tag="pt")
            # weight-major: taps (0,j)+(1,j) contract-128, then (2,j)
            # contract-64; one stationary load per weight.
            for j in range(3):
                for t, (h0, nr) in enumerate(sb):
                    nc.tensor.matmul(pt[:, t, 0:nr * PW],
                                     wb[:, 128 * j:128 * (j + 1)],
                                     xtf[:, h0 * PW + j:h0 * PW + j + nr * PW],
                                     start=(j == 0), stop=False)
            for j in range(3):
                for t, (h0, nr) in enumerate(sb):
                    nc.tensor.matmul(pt[:, t, 0:nr * PW],
                                     wb[0:NCH, 128 * (3 + j):128 * (4 + j)],
                                     xtf[0:NCH,
                                         (h0 + 2) * PW + j:
                                         (h0 + 2) * PW + j + nr * PW],
                                     start=False, stop=(j == 2))
            return pt

        held = {}

        def stage_a(k, pt):
            sb = SBS[k]
            tc_n = len(sb)
            NW = sb[0][1] * PW       # 390 or 130 (singles)
            # cb = (conv + bias ; avg) -> bf16, one 128-partition ACT op
            cb = ep.tile([128, tc_n, NW], BF16, tag="cb")
            nc.scalar.activation(cb, pt[:, 0:tc_n, 0:NW], ACTF.Identity,
                                 bias=cst[:, 0:1], scale=1.0)
            # sq1 = (conv1 + bias1)^2 straight from PSUM
            sq_t = ep.tile([O, tc_n, NW], BF16, tag="sq")
            nc.scalar.activation(sq_t, pt[O:NCH, 0:tc_n, 0:NW], ACTF.Square,
                                 bias=cst[O:NCH, 0:1], scale=1.0)
            # n2m = select((conv0+bias0)^2 + sq1 > b^2, ., BIG)
            n2_t = ep.tile([O, tc_n, NW], BF16, tag="n2")
            nc.vector._custom_dve(
                op_sqsum,
                out=n2_t.rearrange("p a b -> p (a b)"),
                in0=cb[0:O].rearrange("p a b -> p (a b)"),
                in1=sq_t.rearrange("p a b -> p (a b)"),
                s0=0.0, s1=b2, imm2=BIG)
            held[k] = (cb, n2_t)

        def stage_b(k, pt):
            sb = SBS[k]
            tc_n = len(sb)
            NW = sb[0][1] * PW
            cb, n2_t = held.pop(k)
            # r = 1/sqrt(n2m), duplicated to partitions 32-63
            r_t = ep.tile([NCH, tc_n, NW], BF16, tag="r")
            _act_raw(nc, r_t[0:O], n2_t, ACTF.Rsqrt, zb, 1.0)
            nc.vector.tensor_copy(r_t[O:NCH], r_t[0:O])
            # m = cb * r, written to partitions 64-127 so the merge sees
            # both inputs at base partition 64
            m_t = mpool.tile([128, tc_n, NW], BF16, tag="m")
            nc.vector.tensor_mul(m_t[NCH:128], cb[0:NCH], r_t)
            # out = avg + m (Pool mid-kernel; DVE for the last tiny singles)
            ot = outp.tile([NCH, tc_n, NW], BF16, tag="ot")
            eng = nc.gpsimd if k < NSB - 2 else nc.vector
            eng.tensor_add(ot, cb[NCH:128], m_t[NCH:128])
            h0 = sb[0][0]
            nr = sb[0][1]
            tot = tc_n * nr * W
            nc.sync.dma_start(
                out=out_d[:, h0 * W:h0 * W + tot].rearrange(
                    "p (a r c) -> p a r c", a=tc_n, r=nr),
                in_=ot.rearrange("p a (r c) -> p a r c", c=PW)[:, :, :, 0:W])

        NSB = len(SBS)
        pts = [conv(0)]
        for bi in range(NSB):
            if bi + 1 < NSB:
                pts.append(conv(bi + 1))
            stage_a(bi, pts[bi])
            # stage_b of the PREVIOUS batch after this batch's stage_a: its
            # rsqrt input (n2m) is then long ready, so ACT never head-blocks
            if bi >= 1:
                stage_b(bi - 1, pts[bi - 1])
        stage_b(NSB - 1, pts[NSB - 1])

    nc.compile()
    return nc


def _get_nc(b2):
    key = float(b2)
    if key not in _NC:
        _NC[key] = _build_nc(key)
    return _NC[key]


def _prep(params, basis, bias_term, b):
    params = np.asarray(params, np.float32)
    basis = np.asarray(basis, np.float32)
    Kr = np.einsum("abcd,cdefgh->abefgh", params, basis)  # (O,I,K,K,2,2)
    kern = Kr.transpose(0, 4, 1, 5, 2, 3).reshape(2 * O, 2 * I, KS, KS)
    # reference pairs patch (kh=q, kw=p) with kern[o2, c, p, q]:
    Wtap = kern.transpose(0, 1, 3, 2)  # [o2, c, dh, dw]
    # fold per-window mean subtraction into the weights
    Ksum = np.stack([Wtap[:, 0::2].sum(axis=(1, 2, 3)),
                     Wtap[:, 1::2].sum(axis=(1, 2, 3))], axis=1)  # [o2, 2]
    cpar = np.arange(NCH) % 2
    Wp = Wtap - (Ksum[:, cpar] / float(I * KS * KS))[:, :, None, None]
    # device output order: dev channel = 32*v + o  <->  torch channel 2*o + v
    perm = np.array([2 * (i % O) + i // O for i in range(NCH)])
    # avg weights: dev avg row 32*v + o' sums input channels of parity v
    avgw = np.zeros((NCH, NCH), np.float32)   # [avg out row, in ch]
    for v in (0, 1):
        avgw[O * v:O * v + O, v::2] = 1.0 / float(I * KS * KS)
    # blob: [contract row, 6*128 weight cols + 64 id cols]
    blob = np.zeros((128, NBLOB), np.float32)
    for j in range(3):
        # contract-128 weights: row c + 64*s -> tap (s, j)
        for s in (0, 1):
            blob[NCH * s:NCH * s + NCH, 128 * j:128 * j + NCH] = \
                Wp[perm, :, s, j].T
            blob[NCH * s:NCH * s + NCH, 128 * j + NCH:128 * j + 128] = avgw.T
        # contract-64 weights: row c -> tap (2, j)
        blob[0:NCH, 128 * (3 + j):128 * (3 + j) + NCH] = Wp[perm, :, 2, j].T
        blob[0:NCH, 128 * (3 + j) + NCH:128 * (3 + j) + 128] = avgw.T
    bt = np.asarray(bias_term, np.float32).reshape(O, 2)
    cst = np.zeros((128, 2), np.float32)
    for v in (0, 1):
        cst[O * v:O * v + O, 0] = bt[:, v]
    cst[0:O, 1] = bt[:, 1]
    b2 = float(np.asarray(b).reshape(-1)[0]) ** 2
    return blob.astype(ml_dtypes.bfloat16), cst, b2, perm


def _run(inputs, trace=False):
    xx = np.asarray(inputs["xx"], np.float32)
    blob, cst, b2, perm = _prep(inputs["params"], inputs["basis"],
                                inputs["bias_term"], inputs["b"])
    xp = np.pad(xx, ((0, 0), (0, 0), (1, 1), (1, 1)), mode="edge")
    xpb = xp.astype(ml_dtypes.bfloat16)
    in_maps = []
    for core in range(N_CORES):
        bb, half = core // 2, core % 2
        shard = np.ascontiguousarray(xpb[bb, :, half * HS:half * HS + PH, :])
        in_maps.append({"x": shard, "wb": blob, "cst": cst})
    nc = _get_nc(b2)
    res = run_bass_kernel_spmd(nc, in_maps, list(range(N_CORES)), trace=trace)
    out = np.zeros((B, NCH, H, W), np.float32)
    for core in range(N_CORES):
        bb, half = core // 2, core % 2
        dev = np.asarray(res.results[core]["out"]).astype(np.float32)
        out[bb, perm, half * HS:(half + 1) * HS, :] = dev.reshape(NCH, HS, W)
    return out, res.exec_time_ns


def kernel(**inputs):
    out, _ = _run(inputs, trace=False)
    return out
